# revision 53
# baseline (speedup 1.0000x reference)
"""NetVLAD Trainium2 kernel: data-parallel over batch across 8 NeuronCores.

Per-core pipeline (4 images each):
  - x, conv_w passed fp16 from host (halves HBM traffic; ~3.3e-3 rel err)
  - logits = wT.T @ x per 512-pixel chunk; two chunks run concurrently in the
    two column halves of the PE array (tile_position col-tiling)
  - logits transposed to [pixel, k] via PE (fp16); ScalarE bulk-copies the
    transposed logits to SBUF (DVE reads of those PSUM banks wedge the device)
  - per-pixel sumsq on ScalarE (Square+accum from the fp16 PSUM transpose);
    1/||x|| via Newton rsqrt on DVE (avoids activation-table thrash);
    softmax over the free dim with one batched Exp on ScalarE per image
  - x transposed per 128-pixel block (fp16), vlad = ap.T @ xT in bf16,
    asum via interleaved N=1 matmuls
  - pixel blocks processed in group order G so the t/ap tiles line up with
    the [block | block+4] layout the col-tiled logits transposes produce
  - two-stage software pipeline: phase A (DMA/matmuls/transposes/copies) of
    image n+1 is emitted ahead of phase B (softmax tail + vlad + per-image
    norm prep) of image n, so no engine FIFO serializes adjacent images;
    only the tiny global-norm chain and output DMAs run at the end
"""
import sys
sys.path.insert(0, "/opt/trn_rl_repo")
import numpy as np
import concourse.bass as bass
import concourse.tile as tile
from concourse import bacc, mybir, masks
from concourse import bass_utils

F32 = mybir.dt.float32
F16 = mybir.dt.float16
BF16 = mybir.dt.bfloat16
ALU = mybir.AluOpType
ACT = mybir.ActivationFunctionType

N, C, HW, K = 32, 512, 1600, 64
NCORES = 8
NPC = N // NCORES
CT = C // 128                      # 4 c-tiles
NB = 13                            # pixel blocks: 12 x 128 + 1 x 64
BLK = [128] * 12 + [64]
# storage order of blocks: pair-0 transposes yield [j | j+4] double-blocks
G_ORDER = [0, 4, 1, 5, 2, 6, 3, 7, 8, 9, 10, 11, 12]

_CACHE = {}


def _build():
    nc = bacc.Bacc("TRN2", target_bir_lowering=False, debug=False,
                   num_devices=NCORES)
    x_d = nc.dram_tensor("x16", [NPC, C, HW], F16, kind="ExternalInput")
    w_d = nc.dram_tensor("wT16", [C, K], F16, kind="ExternalInput")
    b_d = nc.dram_tensor("conv_b", [1, K], F32, kind="ExternalInput")
    c_d = nc.dram_tensor("centroids", [K, C], F32, kind="ExternalInput")
    y_d = nc.dram_tensor("y", [NPC, K * C], F32, kind="ExternalOutput")
    with tile.TileContext(nc) as tc:
        _emit(nc, tc, x_d, w_d, b_d, c_d, y_d)
    nc.finalize()
    return nc


def _emit(nc, tc, x_d, w_d, b_d, c_d, y_d):
    import contextlib
    ctx = contextlib.ExitStack()
    with ctx:
        const = ctx.enter_context(tc.tile_pool(name="const", bufs=1))
        sb = ctx.enter_context(tc.tile_pool(name="sb", bufs=2))
        ps = ctx.enter_context(tc.tile_pool(name="ps", bufs=1, space="PSUM"))

        # ---- constants ----
        ident16 = const.tile([128, 128], F16)
        masks.make_identity(nc, ident16[:])
        negones = const.tile([1, K], F32)
        nc.vector.memset(negones[:], -1.0)
        onesk = const.tile([K, 1], F32)
        nc.vector.memset(onesk[:], 1.0)
        wT = const.tile([128, CT * K], F16)
        for ct in range(CT):
            nc.sync.dma_start(wT[:, ct * K:(ct + 1) * K],
                              w_d[ct * 128:(ct + 1) * 128, :])
        b_bc = const.tile([128, K], F32)
        nc.sync.dma_start(b_bc[:], b_d[0:1, :].broadcast_to([128, K]))
        b13 = const.tile([128, NB * K], F32)   # conv_b tiled per block group
        for g in range(NB):
            nc.vector.tensor_copy(b13[:, g * K:(g + 1) * K], b_bc[:])
        cent = const.tile([K, C], F32)
        # (cent DMA emitted after image-0 loads; only the epilogue reads it)
        # per-image accumulators held until the deferred epilogue
        vacc = const.tile([64, NPC * C], F32)
        aacc = const.tile([64, NPC], F32)

        def emit_vlad(st):
            vlad_ps = ps.tile([64, C], F32, tag="vlad", name=f"vlad{st['n']}")
            asum_ps = ps.tile([64, 512], F32, tag="asum", name=f"asum{st['n']}")
            for g in range(NB):
                P = BLK[G_ORDER[g]]
                lhsT = st["ap"][0:P, g * K:(g + 1) * K]
                first, last = g == 0, g == NB - 1
                nc.tensor.matmul(
                    vlad_ps[:], lhsT,
                    st["xt_sb"][0:P, g * C:(g + 1) * C],
                    start=first, stop=last)
                nc.tensor.matmul(
                    asum_ps[:, 0:1], lhsT,
                    st["nrm_bf"][0:P, g:g + 1],
                    start=first, stop=last)
            n = st["n"]
            nc.vector.tensor_copy(vacc[:, n * C:(n + 1) * C], vlad_ps[:])
            nc.vector.tensor_copy(aacc[:, n:n + 1], asum_ps[:, 0:1])
            # vl_sb = asum*cent - vlad (negated; sign restored via -g scale)
            vl_sb = sb.tile([64, C], F32, tag="vlsb", bufs=NPC, name=f"vlsb{n}")
            nc.vector.scalar_tensor_tensor(
                out=vl_sb[:], in0=cent[:], scalar=aacc[:, n:n + 1],
                in1=vacc[:, n * C:(n + 1) * C],
                op0=ALU.mult, op1=ALU.subtract)
            scr2 = sb.tile([64, C], F32, tag="scr2", name=f"scr2_{n}")
            nc.vector.scalar_tensor_tensor(
                out=scr2[:], in0=vl_sb[:], scalar=1.0, in1=vl_sb[:],
                op0=ALU.mult, op1=ALU.mult,
                accum_out=ssqc[:, n:n + 1])
            vls.append(vl_sb)

        def emit_A(n):
            xs = [sb.tile([128, HW], F16, tag=f"x{ct}", name=f"x{ct}_{n}")
                  for ct in range(CT)]
            for ct in range(CT):
                nc.sync.dma_start(xs[ct][:], x_d[n, ct * 128:(ct + 1) * 128, :])

            # ---- logits: 2 col-tiled pair matmul groups ----
            lgs = []
            for pi, (la, lb, wb) in enumerate(
                    ((0, 512, 512), (1024, 1536, 64))):
                lg = ps.tile([128, 512], F32, tag="lg", bufs=2,
                             name=f"lg{pi}_{n}")
                for ct in range(CT):
                    nc.tensor.matmul(
                        lg[0:64, :], wT[:, ct * K:(ct + 1) * K],
                        xs[ct][:, la:la + 512],
                        start=(ct == 0), stop=(ct == CT - 1),
                        tile_position=(0, 0))
                for ct in range(CT):
                    nc.tensor.matmul(
                        lg[64:128, 0:wb], wT[:, ct * K:(ct + 1) * K],
                        xs[ct][:, lb:lb + wb],
                        start=(ct == 0), stop=(ct == CT - 1),
                        tile_position=(0, 64))
                lg_sb = sb.tile([128, 512], F16, tag="lgsb", name=f"lgsb{pi}_{n}")
                if pi == 0:
                    nc.scalar.copy(lg_sb[:], lg[:])
                else:
                    nc.scalar.copy(lg_sb[0:64, :], lg[0:64, :])
                    nc.scalar.copy(lg_sb[64:128, 0:64], lg[64:128, 0:64])
                lgs.append(lg_sb)

            # ---- logits transposes (PE) + ScalarE bulk copy to SBUF ----
            lgT0 = ps.tile([128, 1024], F16, tag="lgT", bufs=2,
                           name=f"lgT0_{n}")
            for j in range(4):
                nc.tensor.transpose(
                    lgT0[:, j * 128:(j + 1) * 128],
                    lgs[0][:, j * 128:(j + 1) * 128],
                    ident16[:])
            lgT1 = ps.tile([128, 1024], F16, tag="lgT", bufs=2,
                           name=f"lgT1_{n}")
            for j in range(4):
                nc.tensor.transpose(
                    lgT1[:, j * 64:(j + 1) * 64],
                    lgs[1][0:64, j * 128:(j + 1) * 128],
                    ident16[0:64, 0:64])
            nc.tensor.transpose(
                lgT1[0:64, 256:320],
                lgs[1][64:128, 0:64],
                ident16[64:128, 64:128],
                tile_position=(64, 0))
            # groups 0..7 = blocks [0,4,1,5,2,6,3,7]; groups 8..12 = [8..12]
            lgt0_sb = sb.tile([128, 512], F16, tag="lgt0sb", name=f"lgt0sb{n}")
            nc.scalar.copy(lgt0_sb[:], lgT0[:, 0:512])
            lgt1_sb = sb.tile([128, 320], F16, tag="lgt1sb", name=f"lgt1sb{n}")
            nc.gpsimd.memset(lgt1_sb[64:128, 256:320], 0.0)
            nc.scalar.copy(lgt1_sb[0:128, 0:256], lgT1[:, 0:256])
            nc.scalar.copy(lgt1_sb[0:64, 256:320], lgT1[0:64, 256:320])

            # ---- x transposes + bf16 copy + sumsq, in group order ----
            ss = sb.tile([128, NB], F32, tag="ss", name=f"ss{n}")
            nc.vector.memset(ss[64:128, 12:13], 512.0)
            xt_sb = sb.tile([128, NB * C], BF16, tag="xtsb", name=f"xtsb{n}")
            # two pixel-blocks per (bank-padded) xt tile: one DVE cast per pair
            for g0 in range(0, NB, 2):
                npair = min(2, NB - g0)
                xt = ps.tile([128, 2 * C], F16, tag="xt", bufs=2,
                             name=f"xt{g0}_{n}")
                for h in range(npair):
                    g = g0 + h
                    b = G_ORDER[g]
                    P = BLK[b]
                    for ct in range(CT):
                        nc.tensor.transpose(
                            xt[0:P, h * C + ct * 128:h * C + (ct + 1) * 128],
                            xs[ct][:, b * 128:b * 128 + P], ident16[:])
                    scr = sb.tile([128, C], BF16, tag="scr", name=f"scr{g}_{n}")
                    nc.scalar.activation(scr[0:P, :], xt[0:P, h * C:(h + 1) * C],
                                         ACT.Square,
                                         accum_out=ss[0:P, g:g + 1])
                P0 = BLK[G_ORDER[g0 + npair - 1]]
                if npair == 2 and P0 == 128:
                    nc.vector.tensor_copy(
                        xt_sb[0:128, g0 * C:(g0 + 2) * C], xt[0:128, 0:2 * C])
                else:
                    for h in range(npair):
                        g = g0 + h
                        P = BLK[G_ORDER[g]]
                        nc.vector.tensor_copy(
                            xt_sb[0:P, g * C:(g + 1) * C],
                            xt[0:P, h * C:(h + 1) * C])
            return dict(n=n, ss=ss, xt_sb=xt_sb,
                        lgt0_sb=lgt0_sb, lgt1_sb=lgt1_sb)

        def emit_B(st):
            n = st["n"]
            ss, xt_sb = st["ss"], st["xt_sb"]
            lgt0_sb, lgt1_sb = st["lgt0_sb"], st["lgt1_sb"]
            # ---- inv = 1/sqrt(ss) via Newton on DVE (no act tables) ----
            inv = sb.tile([128, NB], F32, tag="inv", name=f"inv{n}")
            nc.vector.memset(inv[:], 0.044194173824159216)   # 1/sqrt(512)
            ywork = sb.tile([128, 2 * NB], F32, tag="ywork", name=f"yw{n}")
            for it in range(4):
                y2 = ywork[:, 0:NB]
                nc.vector.tensor_mul(y2, inv[:], inv[:])
                h = ywork[:, NB:2 * NB]
                nc.vector.tensor_mul(h, y2, ss[:])
                nc.vector.tensor_scalar(out=h, in0=h, scalar1=-0.5,
                                        scalar2=1.5, op0=ALU.mult,
                                        op1=ALU.add)
                nc.vector.tensor_mul(inv[:], inv[:], h)
            nrm_bf = sb.tile([128, NB], BF16, tag="nrmbf", name=f"nrmbf{n}")
            nc.vector.tensor_mul(nrm_bf[:], ss[:], inv[:])

            # ---- t = lgt*inv + b, batched over the two SBUF regions ----
            t = sb.tile([128, NB * K], F32, tag="t", name=f"t{n}")
            t0v = t[:, 0:512].rearrange("p (s k) -> p s k", s=8)
            nc.vector.tensor_mul(
                t0v, lgt0_sb[:].rearrange("p (s k) -> p s k", s=8),
                inv[:, 0:8].broadcast_to([128, 8, K]))
            nc.vector.tensor_add(t[:, 0:512], t[:, 0:512], b13[:, 0:512])
            t1v = t[:, 512:832].rearrange("p (s k) -> p s k", s=5)
            nc.vector.tensor_mul(
                t1v, lgt1_sb[:].rearrange("p (s k) -> p s k", s=5),
                inv[:, 8:13].broadcast_to([128, 5, K]))
            nc.vector.tensor_add(t[:, 512:832], t[:, 512:832],
                                 b13[:, 512:832])

            # ---- softmax over k (free dim) ----
            tv = t[:].rearrange("p (s k) -> p s k", s=NB)
            negm = sb.tile([128, NB], F32, tag="negm", name=f"negm{n}")
            nc.vector.tensor_reduce(negm[:], tv, axis=mybir.AxisListType.X,
                                    op=ALU.max, negate=True)
            t2 = sb.tile([128, NB * K], F32, tag="t2", name=f"t2{n}")
            t2v = t2[:].rearrange("p (s k) -> p s k", s=NB)
            nc.gpsimd.tensor_add(t2v, tv, negm[:].broadcast_to([128, NB, K]))
            a = sb.tile([128, NB * K], BF16, tag="a", name=f"a{n}")
            nc.scalar.activation(a[:], t2[:], ACT.Exp)
            av = a[:].rearrange("p (s k) -> p s k", s=NB)
            ssum = sb.tile([128, NB], F32, tag="ssum", name=f"ssum{n}")
            nc.vector.tensor_reduce(ssum[:], av, axis=mybir.AxisListType.X,
                                    op=ALU.add)
            rs = sb.tile([128, NB], F32, tag="rs", name=f"rs{n}")
            nc.vector.reciprocal(rs[:], ssum[:])
            rsi = sb.tile([128, NB], F32, tag="rsi", name=f"rsi{n}")
            nc.vector.tensor_mul(rsi[:], rs[:], inv[:])
            ap = sb.tile([128, NB * K], BF16, tag="ap", name=f"ap{n}")
            apv = ap[:].rearrange("p (s k) -> p s k", s=NB)
            nc.gpsimd.tensor_mul(apv, av, rsi[:].broadcast_to([128, NB, K]))

            emit_vlad(dict(n=n, ap=ap, xt_sb=xt_sb, nrm_bf=nrm_bf))

        # software pipeline: front-half of image n+1 runs ahead of the
        # softmax/vlad tail of image n on every engine queue
        vls = []
        ssqc = sb.tile([K, NPC], F32, tag="ssqc", bufs=1)
        sts = [emit_A(0)]
        nc.sync.dma_start(cent[:], c_d[:, :])
        sts.append(emit_A(1))
        for n in range(2, NPC):
            emit_B(sts.pop(0))
            sts.append(emit_A(n))
        for st in sts:
            emit_B(st)

        # ---- deferred epilogue (global norms only) ----
        nc.vector.tensor_scalar_max(ssqc[:], ssqc[:], 1e-24)
        srt = sb.tile([K, NPC], F32, tag="srt", bufs=1)
        nc.scalar.activation(srt[:], ssqc[:], ACT.Sqrt)
        iv = sb.tile([K, NPC], F32, tag="iv", bufs=1)
        nc.vector.reciprocal(iv[:], srt[:])
        contrib = sb.tile([K, NPC], F32, tag="contrib", bufs=1)
        nc.vector.tensor_mul(contrib[:], ssqc[:], iv[:])
        nc.vector.tensor_mul(contrib[:], contrib[:], iv[:])
        totp = ps.tile([1, 512], F32, tag="lg", bufs=2, name="totp")
        nc.tensor.matmul(totp[:, 0:NPC], onesk[:], contrib[:],
                         start=True, stop=True)
        tots = sb.tile([1, NPC], F32, tag="tots", bufs=1)
        nc.scalar.activation(tots[:], totp[:, 0:NPC], ACT.Sqrt)
        g_sb = sb.tile([1, NPC], F32, tag="gsb", bufs=1)
        nc.vector.reciprocal(g_sb[:], tots[:])
        gb = ps.tile([64, 512], F32, tag="xt", bufs=2, name="gb")
        nc.tensor.matmul(gb[:, 0:NPC], negones[:], g_sb[:],
                         start=True, stop=True)
        sc = sb.tile([K, NPC], F32, tag="sc", bufs=1)
        nc.vector.tensor_mul(sc[:], iv[:], gb[:, 0:NPC])
        for n in range(NPC):
            out_sb = sb.tile([K, C], F32, tag="outsb", name=f"out{n}")
            nc.vector.tensor_scalar_mul(out_sb[:], vls[n][:], sc[:, n:n + 1])
            yv = y_d[n:n + 1, :].rearrange("a (k c) -> (a k) c", c=C)
            nc.sync.dma_start(yv, out_sb[:])


def kernel(x, conv_w, conv_b, centroids, trace=False):
    if "nc" not in _CACHE:
        _CACHE["nc"] = _build()
    nc = _CACHE["nc"]
    x16 = np.ascontiguousarray(
        np.asarray(x, dtype=np.float32).reshape(N, C, HW).astype(np.float16))
    wT16 = np.ascontiguousarray(
        np.asarray(conv_w, dtype=np.float32).T.astype(np.float16))
    b = np.asarray(conv_b, dtype=np.float32).reshape(1, K)
    cen = np.asarray(centroids, dtype=np.float32)
    in_maps = []
    for c in range(NCORES):
        in_maps.append({
            "x16": x16[c * NPC:(c + 1) * NPC],
            "wT16": wT16,
            "conv_b": b,
            "centroids": cen,
        })
    res = bass_utils.run_bass_kernel_spmd(nc, in_maps,
                                          core_ids=list(range(NCORES)),
                                          trace=trace)
    out = np.concatenate([res.results[c]["y"] for c in range(NCORES)], axis=0)
    if trace:
        return out, res
    return out
